# revision 13
# baseline (speedup 1.0000x reference)
"""AttentionPool Trainium2 kernel.

Problem: x[B=8, S=4096, D=768] f32; att_v[768]; att_W[768, 768].
  y = tanh(x @ W); scores = y . v; w = softmax(scores over S); out = w . x  -> [B, D]

Sharding: pure data-parallel over batch B — one batch per NeuronCore, 8 cores,
no collectives.

Per-core pipeline (batch b):
  1. SWDGE cast-DMA x_b f32->bf16 into SBUF natural layout [128 s, 768 d] (32 s-tiles)
  2. HWDGE xbar transpose SBUF->SBUF per s-tile: [128 s, 768 d] -> [128 d, 6, 128 s]
  3. PE: y[s-tile] = x_tileT.T @ W  (bf16, psum f32, 12 MMs/tile: 6 k-chunks x {512, 256})
  4. ACT: t = tanh(y_psum) -> bf16 SBUF
  5. DVE: scores[s] = sum_e t*v (tensor_tensor_reduce accum, per-partition = per-s)
  6. ACT (per 8 s-tiles): u = exp(scores) (no max-subtraction needed: |scores| < ~0.5),
     accum_out -> partial Z
  7. PE: p[d] += u_tile.T @ x_tile  (unnormalized pooling, accumulated in PSUM)
  8. out = p / Z  (normalization done on host: Z = sum of the per-partition exp accums)
"""

import sys

sys.path.insert(0, "/opt/trn_rl_repo")

import numpy as np

import concourse.bass as bass
import concourse.mybir as mybir
import concourse.tile as tile
import concourse.tile_sem_assignment as _tsa
from concourse.bass_utils import run_bass_kernel_spmd

# All SWDGE (gpsimd) DMA completions share one semaphore lane. The xbar
# DMA-transpose instruction supports a single HW sync-wait; with 8 lanes a
# transpose needs two waits (its source-tile load + the Tile scheduler's
# xbar-mode serialization against the most recent in-flight copy), which
# walrus rejects ("Too many sync wait commands"). On one lane both
# requirements collapse into a single sem-ge wait.
_tsa.NUM_SWDGE_GLOBAL_SEMS = 1

P = 128
S = 4096
D = 768
NT = S // P  # 32 sequence tiles
DJ = D // P  # 6 contraction chunks
GRP = 8  # s-tiles per exp/pooling group
NG = NT // GRP  # 4 groups
NCORES = 8

F32 = mybir.dt.float32
BF16 = mybir.dt.bfloat16
ACTF = mybir.ActivationFunctionType


def _build(split_waits: bool = True) -> bass.Bass:
    nc = bass.Bass()
    x_d = nc.declare_dram_parameter("x", [S, D], F32, isOutput=False)
    v_d = nc.declare_dram_parameter("att_v", [D], F32, isOutput=False)
    w_d = nc.declare_dram_parameter("att_W", [D, D], F32, isOutput=False)
    p_d = nc.declare_dram_parameter("out_p", [1, D], F32, isOutput=True)
    z_d = nc.declare_dram_parameter("out_z", [P, NG], F32, isOutput=True)

    with tile.TileContext(nc) as tc:
        with (
            tc.tile_pool(name="singles", bufs=1) as singles,
            tc.tile_pool(name="xnat", bufs=NT) as xnat_pool,
            tc.tile_pool(name="xt", bufs=NT) as xt_pool,
            tc.tile_pool(name="tbuf", bufs=3) as t_pool,
            tc.tile_pool(name="sc", bufs=NG) as sc_pool,
            tc.tile_pool(name="ypsum", bufs=2, space="PSUM") as ypsum_pool,
            tc.tile_pool(name="ppsum", bufs=1, space="PSUM") as ppsum_pool,
        ):
            # Weights: W[d, e] with d on partitions, chunked -> [128, 6, 768] bf16
            w_sb = singles.tile([P, DJ, D], BF16)
            nc.gpsimd.dma_start(
                out=w_sb, in_=w_d[:, :].rearrange("(j p) e -> p j e", p=P)
            )
            # v broadcast along partitions: [128, 768] bf16
            v_bc = singles.tile([P, D], BF16)
            nc.gpsimd.dma_start(out=v_bc, in_=v_d[:][None, :].to_broadcast([P, D]))
            # per-group partial Z accumulators (sum over the 8 scores columns,
            # per partition); host sums the 128*4 values.
            zg = singles.tile([P, NG], F32)
            # pooling accumulator psum [1, 768]
            p_ps = ppsum_pool.tile([1, D], F32)

            xn_tiles = []
            scores_g = None
            for i in range(NT):
                # 1. cast-load one s-tile
                xn = xnat_pool.tile([P, D], BF16, name="xn")
                nc.gpsimd.dma_start(out=xn, in_=x_d[i * P : (i + 1) * P, :])
                xn_tiles.append(xn)

                # 2. on-chip xbar transpose -> [128 d, 6, 128 s]
                xt = xt_pool.tile([P, DJ, P], BF16, name="xt")
                nc.sync.dma_start(out=xt, in_=xn, transpose=True)

                # 3. y = xT.T @ W -> psum [128 s, 768 e]
                yps = ypsum_pool.tile([P, D], F32, name="yps")
                for j in range(DJ):
                    nc.tensor.matmul(
                        yps[:, 0:512],
                        lhsT=xt[:, j, :],
                        rhs=w_sb[:, j, 0:512],
                        start=(j == 0),
                        stop=(j == DJ - 1),
                    )
                    nc.tensor.matmul(
                        yps[:, 512:D],
                        lhsT=xt[:, j, :],
                        rhs=w_sb[:, j, 512:D],
                        start=(j == 0),
                        stop=(j == DJ - 1),
                    )

                # 4. t = tanh(y)
                t = t_pool.tile([P, D], BF16, name="t")
                nc.scalar.activation(out=t, in_=yps, func=ACTF.Tanh)

                # 5. scores[:, i%GRP] = sum_e t * v
                if i % GRP == 0:
                    scores_g = sc_pool.tile([P, GRP], F32, name="scores_g")
                dve_out = t_pool.tile([P, D], BF16, name="dve_out")
                # out = (t * 1.0) * v_bc; accum_out = per-partition sum(out)
                nc.vector.scalar_tensor_tensor(
                    out=dve_out,
                    in0=t,
                    scalar=1.0,
                    in1=v_bc,
                    op0=mybir.AluOpType.mult,
                    op1=mybir.AluOpType.mult,
                    accum_out=scores_g[:, (i % GRP) : (i % GRP) + 1],
                )

                # 6+7. per group: u = exp(scores); pool p += u.T @ x
                if i % GRP == GRP - 1:
                    g = i // GRP
                    u_g = sc_pool.tile([P, GRP], BF16, name="u_g")
                    nc.scalar.activation(
                        out=u_g,
                        in_=scores_g,
                        func=ACTF.Exp,
                        accum_out=zg[:, g : g + 1],
                    )
                    for k in range(GRP):
                        ii = g * GRP + k
                        xsrc = xn_tiles[ii]
                        nc.tensor.matmul(
                            p_ps[:, 0:512],
                            lhsT=u_g[:, k : k + 1],
                            rhs=xsrc[:, 0:512],
                            start=(ii == 0),
                            stop=(ii == NT - 1),
                            skip_group_check=True,
                        )
                        nc.tensor.matmul(
                            p_ps[:, 512:D],
                            lhsT=u_g[:, k : k + 1],
                            rhs=xsrc[:, 512:D],
                            start=(ii == 0),
                            stop=(ii == NT - 1),
                            skip_group_check=True,
                        )

            # 8. write out unnormalized p and the Z partials
            p_sb = singles.tile([1, D], F32)
            nc.scalar.copy(out=p_sb, in_=p_ps)
            nc.sync.dma_start(out=p_d[:, :], in_=p_sb)
            nc.sync.dma_start(out=z_d[:, :], in_=zg)

    if split_waits:
        _split_excess_transpose_waits(nc)
    return nc


def _split_excess_transpose_waits(nc: bass.Bass) -> None:
    """DMA instructions (PSEUDO_DMA_DIRECT2D / DMA_DIRECT2D_XPOSE) carry a
    single HW sync-wait slot; Tile can attach more (source dep + DMA-lane
    reuse + xbar-mode serialization). Move all but one wait onto
    InstEventSemaphore(s) inserted just before, on the same engine — the
    sequencer executes waits in order, so semantics are unchanged."""
    fn = nc.m.functions[0]
    for blk in fn.blocks:
        insts = blk.instructions
        new_insts = []
        for inst in insts:
            si = inst.sync_info
            if (
                not isinstance(inst, mybir.InstEventSemaphore)
                and si is not None
                and len(si.on_wait) > 1
            ):
                waits = list(si.on_wait)
                for w in waits[:-1]:
                    ev = mybir.InstEventSemaphore(
                        name=nc.get_next_instruction_name(), ins=[], outs=[]
                    )
                    ev.engine = inst.engine
                    ev.sync_info = mybir.SyncInfo(on_wait=[w], on_update=[])
                    new_insts.append(ev)
                inst.sync_info = mybir.SyncInfo(
                    on_wait=waits[-1:], on_update=list(si.on_update)
                )
            new_insts.append(inst)
        blk.instructions = new_insts


_CACHE: dict = {}
LAST_RESULT = None


def _get_nc() -> bass.Bass:
    if "nc" not in _CACHE:
        _CACHE["nc"] = _build()
    return _CACHE["nc"]


def kernel(x: np.ndarray, att_v: np.ndarray, att_W: np.ndarray) -> np.ndarray:
    global LAST_RESULT
    assert x.shape == (NCORES, S, D), x.shape
    nc = _get_nc()
    in_maps = [
        {
            "x": np.ascontiguousarray(x[b], dtype=np.float32),
            "att_v": np.ascontiguousarray(att_v, dtype=np.float32),
            "att_W": np.ascontiguousarray(att_W, dtype=np.float32),
        }
        for b in range(NCORES)
    ]
    res = run_bass_kernel_spmd(nc, in_maps, core_ids=list(range(NCORES)))
    LAST_RESULT = res
    outs = []
    for b in range(NCORES):
        p = res.results[b]["out_p"][0]
        z = res.results[b]["out_z"].sum(dtype=np.float64)
        outs.append(p / z)
    return np.stack(outs).astype(np.float32)


# revision 15
# speedup vs baseline: 1.4448x; 1.4448x over previous
"""AttentionPool Trainium2 kernel.

Problem: x[B=8, S=4096, D=768] f32; att_v[768]; att_W[768, 768].
  y = tanh(x @ W); scores = y . v; w = softmax(scores over S); out = w . x  -> [B, D]

Sharding: pure data-parallel over batch B — one batch per NeuronCore, 8 cores,
no collectives.

Per-core pipeline (batch b):
  1. SWDGE cast-DMA x_b f32->bf16 into SBUF natural layout [128 s, 768 d] (32 s-tiles)
  2. HWDGE xbar transpose SBUF->SBUF per s-tile: [128 s, 768 d] -> [128 d, 6, 128 s]
  3. PE: y[s-tile] = x_tileT.T @ W  (bf16, psum f32, 12 MMs/tile: 6 k-chunks x {512, 256})
  4. ACT: t = tanh(y_psum) -> bf16 SBUF
  5. DVE: scores[s] = sum_e t*v (tensor_tensor_reduce accum, per-partition = per-s)
  6. ACT (per 8 s-tiles): u = exp(scores) (no max-subtraction needed: |scores| < ~0.5),
     accum_out -> partial Z
  7. PE: p[d] += u_tile.T @ x_tile  (unnormalized pooling, accumulated in PSUM)
  8. out = p / Z  (normalization done on host: Z = sum of the per-partition exp accums)
"""

import sys

sys.path.insert(0, "/opt/trn_rl_repo")

import numpy as np

import concourse.bass as bass
import concourse.mybir as mybir
import concourse.tile as tile
import concourse.tile_sem_assignment as _tsa
from concourse.bass_utils import run_bass_kernel_spmd
from concourse.masks import make_identity

# All SWDGE (gpsimd) DMA completions share one semaphore lane. The xbar
# DMA-transpose instruction supports a single HW sync-wait; with 8 lanes a
# transpose needs two waits (its source-tile load + the Tile scheduler's
# xbar-mode serialization against the most recent in-flight copy), which
# walrus rejects ("Too many sync wait commands"). On one lane both
# requirements collapse into a single sem-ge wait.
_tsa.NUM_SWDGE_GLOBAL_SEMS = 1

P = 128
S = 4096
D = 768
NT = S // P  # 32 sequence tiles
DJ = D // P  # 6 contraction chunks
GRP = 8  # s-tiles per exp/pooling group
NG = NT // GRP  # 4 groups
NCORES = 8

F32 = mybir.dt.float32
BF16 = mybir.dt.bfloat16
ACTF = mybir.ActivationFunctionType


def _build(split_waits: bool = True) -> bass.Bass:
    nc = bass.Bass()
    x_d = nc.declare_dram_parameter("x", [S, D], F32, isOutput=False)
    v_d = nc.declare_dram_parameter("att_v", [D], F32, isOutput=False)
    w_d = nc.declare_dram_parameter("att_W", [D, D], F32, isOutput=False)
    p_d = nc.declare_dram_parameter("out_p", [1, D], F32, isOutput=True)
    z_d = nc.declare_dram_parameter("out_z", [P, NG], F32, isOutput=True)

    with tile.TileContext(nc) as tc:
        with (
            tc.tile_pool(name="singles", bufs=1) as singles,
            tc.tile_pool(name="xnat", bufs=NT) as xnat_pool,
            tc.tile_pool(name="xt", bufs=NT) as xt_pool,
            tc.tile_pool(name="tbuf", bufs=3) as t_pool,
            tc.tile_pool(name="sc", bufs=NG) as sc_pool,
            tc.tile_pool(name="ypsum", bufs=2, space="PSUM") as ypsum_pool,
            tc.tile_pool(name="xtpsum", bufs=2, space="PSUM") as xtp_pool,
            tc.tile_pool(name="ppsum", bufs=1, space="PSUM") as ppsum_pool,
        ):
            # v broadcast along partitions: [128, 768] bf16 (first in the
            # SWDGE FIFO — tiny, doesn't delay the x loads)
            v_bc = singles.tile([P, D], BF16)
            nc.gpsimd.dma_start(out=v_bc, in_=v_d[:][None, :].to_broadcast([P, D]))
            # identity for PE transpose-mode
            ident = singles.tile([P, P], BF16)
            make_identity(nc, ident)
            # Weights: W[d, e] with d on partitions, chunked -> [128, 6, 768]
            # bf16. Loaded as 6 per-chunk DMAs interleaved after the first x
            # tile so the first transposes / matmuls aren't gated on the
            # whole 2.25 MB of W in the single SWDGE completion FIFO.
            w_sb = singles.tile([P, DJ, D], BF16)
            # per-group partial Z accumulators (sum over the 8 scores columns,
            # per partition); host sums the 128*4 values.
            zg = singles.tile([P, NG], F32)
            # pooling accumulator psum [1, 768]
            p_ps = ppsum_pool.tile([1, D], F32)

            xn_tiles = []
            scores_g = None
            for i in range(NT):
                # 1. cast-load one s-tile
                xn = xnat_pool.tile([P, D], BF16, name="xn")
                nc.gpsimd.dma_start(out=xn, in_=x_d[i * P : (i + 1) * P, :])
                xn_tiles.append(xn)
                if i == 0:
                    for j in range(DJ):
                        nc.gpsimd.dma_start(
                            out=w_sb[:, j, :], in_=w_d[j * P : (j + 1) * P, :]
                        )

                # 2. PE transpose-mode: x tile -> [128 d, 6*128 s] via PSUM
                xt_ps = xtp_pool.tile([P, D], BF16, name="xt_ps")
                for j in range(DJ):
                    nc.tensor.transpose(
                        xt_ps[:, j * P : (j + 1) * P],
                        xn[:, j * P : (j + 1) * P],
                        ident,
                    )
                xt = xt_pool.tile([P, D], BF16, name="xt")
                nc.vector.tensor_copy(out=xt, in_=xt_ps)

                # 3. y = xT.T @ W -> psum [128 s, 768 e]
                yps = ypsum_pool.tile([P, D], F32, name="yps")
                for j in range(DJ):
                    nc.tensor.matmul(
                        yps[:, 0:512],
                        lhsT=xt[:, j * P : (j + 1) * P],
                        rhs=w_sb[:, j, 0:512],
                        start=(j == 0),
                        stop=(j == DJ - 1),
                    )
                    nc.tensor.matmul(
                        yps[:, 512:D],
                        lhsT=xt[:, j * P : (j + 1) * P],
                        rhs=w_sb[:, j, 512:D],
                        start=(j == 0),
                        stop=(j == DJ - 1),
                    )

                # 4. t = tanh(y)
                t = t_pool.tile([P, D], BF16, name="t")
                nc.scalar.activation(out=t, in_=yps, func=ACTF.Tanh)

                # 5. scores[:, i%GRP] = sum_e t * v
                if i % GRP == 0:
                    scores_g = sc_pool.tile([P, GRP], F32, name="scores_g")
                dve_out = t_pool.tile([P, D], BF16, name="dve_out")
                # out = (t * 1.0) * v_bc; accum_out = per-partition sum(out)
                nc.vector.scalar_tensor_tensor(
                    out=dve_out,
                    in0=t,
                    scalar=1.0,
                    in1=v_bc,
                    op0=mybir.AluOpType.mult,
                    op1=mybir.AluOpType.mult,
                    accum_out=scores_g[:, (i % GRP) : (i % GRP) + 1],
                )

                # 6+7. per group: u = exp(scores); pool p += u.T @ x
                if i % GRP == GRP - 1:
                    g = i // GRP
                    u_g = sc_pool.tile([P, GRP], BF16, name="u_g")
                    nc.scalar.activation(
                        out=u_g,
                        in_=scores_g,
                        func=ACTF.Exp,
                        accum_out=zg[:, g : g + 1],
                    )
                    for k in range(GRP):
                        ii = g * GRP + k
                        xsrc = xn_tiles[ii]
                        nc.tensor.matmul(
                            p_ps[:, 0:512],
                            lhsT=u_g[:, k : k + 1],
                            rhs=xsrc[:, 0:512],
                            start=(ii == 0),
                            stop=(ii == NT - 1),
                            skip_group_check=True,
                        )
                        nc.tensor.matmul(
                            p_ps[:, 512:D],
                            lhsT=u_g[:, k : k + 1],
                            rhs=xsrc[:, 512:D],
                            start=(ii == 0),
                            stop=(ii == NT - 1),
                            skip_group_check=True,
                        )

            # 8. write out unnormalized p and the Z partials
            p_sb = singles.tile([1, D], F32)
            nc.scalar.copy(out=p_sb, in_=p_ps)
            nc.sync.dma_start(out=p_d[:, :], in_=p_sb)
            nc.sync.dma_start(out=z_d[:, :], in_=zg)

    if split_waits:
        _split_excess_transpose_waits(nc)
    return nc


def _split_excess_transpose_waits(nc: bass.Bass) -> None:
    """DMA instructions (PSEUDO_DMA_DIRECT2D / DMA_DIRECT2D_XPOSE) carry a
    single HW sync-wait slot; Tile can attach more (source dep + DMA-lane
    reuse + xbar-mode serialization). Move all but one wait onto
    InstEventSemaphore(s) inserted just before, on the same engine — the
    sequencer executes waits in order, so semantics are unchanged."""
    fn = nc.m.functions[0]
    for blk in fn.blocks:
        insts = blk.instructions
        new_insts = []
        for inst in insts:
            si = inst.sync_info
            if (
                not isinstance(inst, mybir.InstEventSemaphore)
                and si is not None
                and len(si.on_wait) > 1
            ):
                waits = list(si.on_wait)
                for w in waits[:-1]:
                    ev = mybir.InstEventSemaphore(
                        name=nc.get_next_instruction_name(), ins=[], outs=[]
                    )
                    ev.engine = inst.engine
                    ev.sync_info = mybir.SyncInfo(on_wait=[w], on_update=[])
                    new_insts.append(ev)
                inst.sync_info = mybir.SyncInfo(
                    on_wait=waits[-1:], on_update=list(si.on_update)
                )
            new_insts.append(inst)
        blk.instructions = new_insts


_CACHE: dict = {}
LAST_RESULT = None


def _get_nc() -> bass.Bass:
    if "nc" not in _CACHE:
        _CACHE["nc"] = _build()
    return _CACHE["nc"]


def kernel(x: np.ndarray, att_v: np.ndarray, att_W: np.ndarray) -> np.ndarray:
    global LAST_RESULT
    assert x.shape == (NCORES, S, D), x.shape
    nc = _get_nc()
    in_maps = [
        {
            "x": np.ascontiguousarray(x[b], dtype=np.float32),
            "att_v": np.ascontiguousarray(att_v, dtype=np.float32),
            "att_W": np.ascontiguousarray(att_W, dtype=np.float32),
        }
        for b in range(NCORES)
    ]
    res = run_bass_kernel_spmd(nc, in_maps, core_ids=list(range(NCORES)))
    LAST_RESULT = res
    outs = []
    for b in range(NCORES):
        p = res.results[b]["out_p"][0]
        z = res.results[b]["out_z"].sum(dtype=np.float64)
        outs.append(p / z)
    return np.stack(outs).astype(np.float32)
